# revision 11
# baseline (speedup 1.0000x reference)
"""Multi-Head Latent Attention (MLA) Trainium2 kernel — v4.

Problem (hardcoded): B=2, S=2048, D_MODEL=2048, H=16, HEAD_DIM=128,
D_LATENT=512 (D_QK=256 / D_V=256), ROPE_DIM=64, fp32 in/out.

Reference semantics: q = concat([q_no_rope(1024), q_rope(1024)]).reshape(16
heads x 128), so heads 0-7 take both 64-dim halves from the latent
decompression and heads 8-15 take both halves from the rope projection of x;
RoPE rotates dims 64:128 of every head.

Sharding: 8 cores = 2 batches x 4 head-groups; core (b, hg) owns heads
[2hg, 2hg+1, 8+2hg, 8+2hg+1] (2 decompression + 2 rope-projection heads),
computes the shared latent for its batch redundantly, and produces a partial
output projection (its heads' rows of W_out), transposed [e, q]. The host
sums the 4 partials per batch (in f32; device emits bf16 partials).

v4 design notes (from v2=351.5us / v3=360.8us profiles):
  - PE matmuls stream at 215ns issue-to-issue; the kernel floor is ~292us
    of matmul issue + preamble/DMA ramp + drain. Everything here is about
    keeping PE issuing continuously.
  - barrier-free phase transition: stage-1 pools (xt/wbig/ps1) are freed
    right after stage-1 emission (their only consumers are stage-1 ops, so
    the release barrier is cheap), the attention pools are allocated
    immediately, and the v decompression runs inside them (psums from the
    ps_s pool). v3 lost ~8us at this boundary waiting on queued DVE rope.
  - rope processes head PAIRS with full-width [128,S] DVE ops (v2/v3 used
    [64,S] ops at half DVE rate): stack the swapped halves of two heads in
    one scratch tile (4 DMAs), one sin-mul, one cos-mul (raw halves also
    stacked via 2 DMAs), one add, then 2 SBUF->SBUF DMAs write the roped
    rows back. 3.6us DVE per pair instead of 7.2us.
  - softmax denominator: DVE add-tree emitted eagerly inside the kc stream
    (lvl3 right after the last lvl2), so the bf16 ones-colsum matmul at
    group end never waits on DVE. v3 deferred the tree a full group and
    stalled ~0.6us/group on it.
  - single weave queue of PE filler units popped one per kc slot (and
    before the first scores of each group): 4 held-back v blocks + the 16
    q/k decompression pairs (groups 0-3, meeting the rope deadlines for
    the dec-head groups at g4/g5), then each q-chunk's 16 out-proj m-tiles
    as it completes (cap 5/group keeps every later group PE-bound).
  - all attention-phase psum->SBUF copies are DVE CASTs; ACT does exps
    only (584ns/kc is the ACT-bound cadence, PE must supply >=2 fill
    matmuls per kc to stay the bottleneck).

On-chip layout is feature-major so matmuls contract over partitions.
"""

import math

import numpy as np

B = 2
S = 2048
D = 2048
H4 = 4            # heads per core
HD = 128          # head dim
DL = 512          # d_latent
DQK = 256
RD = 64           # rope dim
NC = 8            # cores

SCALE = 1.0 / math.sqrt(HD)

_prog_cache = {}


def _build_program(phases=4):
    import concourse.tile as tile
    from concourse import bacc, mybir

    bf16 = mybir.dt.bfloat16
    f32 = mybir.dt.float32

    nc = bacc.Bacc("TRN2", target_bir_lowering=False, debug=False, num_devices=1)

    xT = nc.dram_tensor("xT", [D, S], bf16, kind="ExternalInput")
    w_big = nc.dram_tensor("w_big", [D, 1024], bf16, kind="ExternalInput")
    w_qk = nc.dram_tensor("w_qk", [DQK, 512], bf16, kind="ExternalInput")
    w_v = nc.dram_tensor("w_v", [DQK, 512], bf16, kind="ExternalInput")
    w_o = nc.dram_tensor("w_o", [DL, D], bf16, kind="ExternalInput")
    cos4_d = nc.dram_tensor("cos4", [128, S], bf16, kind="ExternalInput")
    sin4w_d = nc.dram_tensor("sin4w", [128, S], bf16, kind="ExternalInput")
    out_d = nc.dram_tensor("out", [D, S], bf16, kind="ExternalOutput")

    NQ = S // 512    # 4 q chunks of 512 (attention)
    NK = S // 128    # 16 k/seq chunks of 128
    KD = D // 128    # 16 contraction chunks for stage 1
    N4 = S // 1024   # 2 wide n-chunks of 1024 (stage1)

    with tile.TileContext(nc, pool_alloc_mode="queue") as tc:
        import contextlib

        with contextlib.ExitStack() as ctx:
            # persistent pools (live to end of program, LIFO via ExitStack)
            ones_p = ctx.enter_context(tc.tile_pool(name="onesp", bufs=1))
            qk_p = ctx.enter_context(tc.tile_pool(name="qk", bufs=1))
            v_p = ctx.enter_context(tc.tile_pool(name="vp", bufs=1))

            ones_f32 = ones_p.tile([128, 128], f32)
            nc.gpsimd.memset(ones_f32[:], 1.0)
            ones_bf = ones_p.tile([128, 128], bf16)
            nc.vector.tensor_copy(ones_bf[:], ones_f32[:])
            # per-head assembled q/k: rows 0:64 nr dims, 64:128 roped dims
            # qkT[0..3] = q heads 0..3, qkT[4..7] = k heads 0..3
            qkT = [qk_p.tile([128, S], bf16, name=f"qkT{i}", tag=f"qk{i}")
                   for i in range(8)]
            v_nat = [v_p.tile([128, 512], bf16, name=f"v{i}", tag=f"v{i}")
                     for i in range(NK)]

            consts_cm = tc.tile_pool(name="consts", bufs=1)
            consts = consts_cm.__enter__()
            swp_cm = tc.tile_pool(name="swpA", bufs=2)
            swp_p = swp_cm.__enter__()
            scr_cm = tc.tile_pool(name="scrA", bufs=3)
            scr_p = scr_cm.__enter__()

            # wdec/lat outlive the stage-1 pools (the weave dec units read
            # them mid-attention) — right-side stack.
            wdec_cm = tc.tile_pool(name="wdec", bufs=1, side="right")
            wdec_p = wdec_cm.__enter__()
            lat_cm = tc.tile_pool(name="lat", bufs=1, side="right")
            lat_p = lat_cm.__enter__()
            # latn[l][n4]: latent rows l*128:(l+1)*128, cols n4*1024:+1024
            latn = [[lat_p.tile([128, 1024], bf16, name=f"latT{i}_{n}",
                                tag=f"lat{i}_{n}") for n in range(N4)]
                    for i in range(4)]

            # ---------------- stage 1: bigT = w_big^T @ xT -----------------
            # single pass; w_big fully resident (bf16) loaded on the Scalar
            # HWDGE queue, xT read once as [128,1024] tiles on the Sync
            # queue (two queues run in parallel); per 512-wide n-chunk all
            # 8 m-tiles accumulate in 8 psum banks.
            ps1_cm = tc.tile_pool(name="ps1", bufs=8, space="PSUM")
            ps1_p = ps1_cm.__enter__()
            wbig_cm = tc.tile_pool(name="wbig", bufs=1)
            wbig_p = wbig_cm.__enter__()
            xt_cm = tc.tile_pool(name="xt", bufs=18)
            xt_p = xt_cm.__enter__()

            wbig_sb = [wbig_p.tile([128, 1024], bf16, name=f"wb{k}",
                                   tag=f"wb{k}") for k in range(KD)]
            for n4 in range(N4):
                xts = []
                for k in range(KD):
                    if n4 == 0:
                        nc.scalar.dma_start(wbig_sb[k][:],
                                            w_big.ap()[k * 128:(k + 1) * 128, :])
                    x_t = xt_p.tile([128, 1024], bf16, name="xt", tag="xt")
                    nc.sync.dma_start(
                        x_t[:], xT.ap()[k * 128:(k + 1) * 128,
                                        n4 * 1024:(n4 + 1) * 1024])
                    xts.append(x_t)
                if n4 == 0:
                    # phase-2 + attention weights + rope consts on the
                    # scalar HWDGE queue behind the w_big tiles (they never
                    # delay the xT stream).
                    wqk_sb = []
                    for l in range(2):
                        w_t = wdec_p.tile([128, 512], bf16, name=f"wqk{l}",
                                          tag=f"wqk{l}")
                        nc.scalar.dma_start(w_t[:],
                                            w_qk.ap()[l * 128:(l + 1) * 128, :])
                        wqk_sb.append(w_t)
                    wv_sb = []
                    for l in range(2):
                        w_t = wdec_p.tile([128, 512], bf16, name=f"wv{l}",
                                          tag=f"wv{l}")
                        nc.scalar.dma_start(w_t[:],
                                            w_v.ap()[l * 128:(l + 1) * 128, :])
                        wv_sb.append(w_t)
                    cos4 = consts.tile([128, S], bf16)
                    nc.scalar.dma_start(cos4[:], cos4_d.ap()[:])
                    sin4w = consts.tile([128, S], bf16)
                    nc.scalar.dma_start(sin4w[:], sin4w_d.ap()[:])
                for sub in range(2):
                    psums = [ps1_p.tile([128, 512], f32, name=f"ps1_{m}",
                                        tag="ps1") for m in range(8)]
                    for k in range(KD):
                        for m in range(8):
                            nc.tensor.matmul(
                                psums[m][:],
                                wbig_sb[k][:, m * 128:(m + 1) * 128],
                                xts[k][:, sub * 512:(sub + 1) * 512],
                                start=(k == 0),
                                stop=(k == KD - 1),
                            )
                    lsl = slice(sub * 512, (sub + 1) * 512)
                    nsl = slice(n4 * 1024 + sub * 512,
                                n4 * 1024 + (sub + 1) * 512)
                    for m in range(8):
                        if m < 4:
                            nc.vector.tensor_copy(latn[m][n4][:, lsl],
                                                  psums[m][:])
                        else:
                            dst = qkT[[2, 3, 6, 7][m - 4]]
                            nc.vector.tensor_copy(dst[:, nsl], psums[m][:])

            # exp-table warm-up on the scalar queue behind the DMA issues
            warm = ones_p.tile([128, 1], f32)
            nc.scalar.activation(warm[:], ones_f32[:, 0:1],
                                 mybir.ActivationFunctionType.Exp)

            if phases == 1:
                for i in range(4):
                    for n in range(N4):
                        nc.sync.dma_start(
                            out_d.ap()[i * 128:(i + 1) * 128,
                                       n * 1024:(n + 1) * 1024],
                            latn[i][n][:])
                for i, t in enumerate(qkT):
                    nc.sync.dma_start(
                        out_d.ap()[512 + i * 128:512 + (i + 1) * 128, :], t[:])

            # ---------------- rope helper (head pair, full-width DVE) ------
            # roped rows t[64:128] = raw*cos + swap32(raw)*sin for two head
            # tiles at once: swapped halves and raw halves of BOTH tiles are
            # stacked into [128,S] scratch (DMA), three full-width DVE ops
            # compute the result, and two DMAs write the rows back.
            def rope_pair(i, j):
                ti, tj = qkT[i], qkT[j]
                sw = swp_p.tile([128, S], bf16, name=f"sw{i}_{j}", tag="sw")
                nc.sync.dma_start(sw[0:32, :], ti[96:128, :])
                nc.sync.dma_start(sw[32:64, :], ti[64:96, :])
                nc.sync.dma_start(sw[64:96, :], tj[96:128, :])
                nc.sync.dma_start(sw[96:128, :], tj[64:96, :])
                raw = scr_p.tile([128, S], bf16, name="raw", tag="scr")
                nc.sync.dma_start(raw[0:64, :], ti[64:128, :])
                nc.sync.dma_start(raw[64:128, :], tj[64:128, :])
                tsin = scr_p.tile([128, S], bf16, name="tsin", tag="scr")
                nc.vector.tensor_mul(tsin[:], sw[:], sin4w[:])
                res = scr_p.tile([128, S], bf16, name="res", tag="scr")
                nc.vector.tensor_mul(res[:], raw[:], cos4[:])
                nc.vector.tensor_add(res[:], res[:], tsin[:])
                nc.sync.dma_start(ti[64:128, :], res[0:64, :])
                nc.sync.dma_start(tj[64:128, :], res[64:128, :])

            # debug-path rope (serial, in-place; matches v2)
            def rope_tiles_dbg(idxs):
                for i in idxs:
                    t = qkT[i]
                    sw = swp_p.tile([64, S], bf16, name=f"swd{i}", tag="sw")
                    nc.sync.dma_start(sw[0:32, :], t[96:128, :])
                    nc.sync.dma_start(sw[32:64, :], t[64:96, :])
                    tmp_sin = scr_p.tile([64, S], bf16, name="tsd", tag="scr")
                    nc.vector.tensor_mul(tmp_sin[0:64, :], sw[0:64, :],
                                         sin4w[0:64, :])
                    tmp_cos = scr_p.tile([64, S], bf16, name="tcd", tag="scr")
                    nc.vector.tensor_mul(tmp_cos[0:64, :], t[64:128, :],
                                         cos4[64:128, :])
                    nc.vector.tensor_add(t[64:128, :], tmp_cos[0:64, :],
                                         tmp_sin[0:64, :])

            if phases == 2 or phases == 3:
                # serial phase-2 (v + dec) + dumps, pre-attention layout
                for sc in range(NK):
                    ps = ps1_p.tile([128, 512], f32, name="ps2v", tag="ps1")
                    for l in range(2):
                        nc.tensor.matmul(
                            ps[:],
                            latn[2 + l][sc // 8][:, (sc % 8) * 128:
                                                 (sc % 8 + 1) * 128],
                            wv_sb[l][:],
                            start=(l == 0), stop=(l == 1),
                        )
                    nc.vector.tensor_copy(v_nat[sc][:], ps[:])
                for mt in [0, 2, 1, 3]:
                    for n in range(NQ):
                        nsl = slice(n * 512, (n + 1) * 512)
                        ps = ps1_p.tile([128, 512], f32, name="ps2",
                                        tag="ps1")
                        for l in range(2):
                            nc.tensor.matmul(
                                ps[:],
                                wqk_sb[l][:, mt * 128:(mt + 1) * 128],
                                latn[l][n // 2][:, (n % 2) * 512:
                                                (n % 2 + 1) * 512],
                                start=(l == 0), stop=(l == 1),
                            )
                        nc.vector.tensor_copy(qkT[[0, 1, 4, 5][mt]][:, nsl],
                                              ps[:])
                rope_tiles_dbg([2, 6, 3, 7, 0, 4, 1, 5])
                for i, t in enumerate(qkT):
                    nc.sync.dma_start(out_d.ap()[i * 128:(i + 1) * 128, :],
                                      t[:])
                if phases == 3:
                    for sc in range(NK):
                        nc.sync.dma_start(
                            out_d.ap()[1024 + (sc // 4) * 128:
                                       1024 + (sc // 4 + 1) * 128,
                                       (sc % 4) * 512:(sc % 4 + 1) * 512],
                            v_nat[sc][:])

            # free the stage-1 pools NOW: their only consumers are stage-1
            # ops, so the release barrier is cheap, and the attention pools
            # take over the space with no mid-phase barrier.
            xt_cm.__exit__(None, None, None)
            wbig_cm.__exit__(None, None, None)
            ps1_cm.__exit__(None, None, None)

            # ---------------- attention + v-dec + output projection --------
            if phases >= 4:
              with tc.tile_pool(name="wo", bufs=1) as wo_p, \
                 tc.tile_pool(name="exp", bufs=19) as exp_p, \
                 tc.tile_pool(name="den1", bufs=9) as den1_p, \
                 tc.tile_pool(name="den2", bufs=5) as den2_p, \
                 tc.tile_pool(name="den3", bufs=3) as den3_p, \
                 tc.tile_pool(name="acc", bufs=2) as acc_p, \
                 tc.tile_pool(name="ctx", bufs=9) as ctx_p, \
                 tc.tile_pool(name="rden", bufs=2) as rden_p, \
                 tc.tile_pool(name="stage", bufs=4) as stage_p, \
                 tc.tile_pool(name="ps_s", bufs=3, space="PSUM") as ps_s_p, \
                 tc.tile_pool(name="ps_c", bufs=2, space="PSUM") as ps_c_p, \
                 tc.tile_pool(name="ps_o", bufs=3, space="PSUM") as ps_o_p:
                wo_sb = []
                for kk in range(4):
                    w_t = wo_p.tile([128, D], bf16, name=f"wo{kk}",
                                    tag=f"wo{kk}")
                    nc.scalar.dma_start(w_t[:],
                                        w_o.ap()[kk * 128:(kk + 1) * 128, :])
                    wo_sb.append(w_t)

                # rope of the x-projection heads: [2,6] needed by group 0,
                # overlaps the v loop on DVE; [3,7] needed by group 1,
                # overlaps group 0.
                rope_pair(2, 6)

                # v decompression (kc 0..11 serial; 12..15 are weave units
                # inside group 0, whose avs reach kc=12 much later)
                def emit_v(sc, copy_eng, pool=None):
                    ps = (pool or ps_s_p).tile([128, 512], f32, name="psv",
                                               tag=("pso" if pool else "pss"))
                    for l in range(2):
                        nc.tensor.matmul(
                            ps[:],
                            latn[2 + l][sc // 8][:, (sc % 8) * 128:
                                                 (sc % 8 + 1) * 128],
                            wv_sb[l][:],
                            start=(l == 0), stop=(l == 1),
                        )
                    if copy_eng is nc.scalar:
                        copy_eng.copy(v_nat[sc][:], ps[:])
                    else:
                        copy_eng.tensor_copy(v_nat[sc][:], ps[:])

                for sc in range(12):
                    # ACT is otherwise idle here; DVE is roping [2,6]
                    emit_v(sc, nc.scalar if sc % 3 else nc.vector)

                rope_pair(3, 7)

                # ---- weave queue: PE filler units, one popped per slot ----
                weave_q = []

                def weave(n=1):
                    for _ in range(n):
                        if weave_q:
                            weave_q.pop(0)()

                def mk_v_unit(sc):
                    def emit():
                        emit_v(sc, nc.vector, pool=ps_o_p)
                    return emit

                def mk_dec_unit(mt, n):
                    def emit():
                        nsl = slice(n * 512, (n + 1) * 512)
                        ps = ps_o_p.tile([128, 512], f32, name="pso",
                                         tag="pso")
                        for l in range(2):
                            nc.tensor.matmul(
                                ps[:],
                                wqk_sb[l][:, mt * 128:(mt + 1) * 128],
                                latn[l][n // 2][:, (n % 2) * 512:
                                                (n % 2 + 1) * 512],
                                start=(l == 0), stop=(l == 1),
                            )
                        nc.vector.tensor_copy(qkT[[0, 1, 4, 5][mt]][:, nsl],
                                              ps[:])
                    return emit

                def mk_out_unit(qc, ctx_by_head, m):
                    def emit():
                        qsl = slice(qc * 512, (qc + 1) * 512)
                        ps_o = ps_o_p.tile([128, 512], f32, name="pso",
                                           tag="pso")
                        for kk in range(4):
                            nc.tensor.matmul(
                                ps_o[:],
                                wo_sb[kk][:, m * 128:(m + 1) * 128],
                                ctx_by_head[kk][:],
                                start=(kk == 0), stop=(kk == 3),
                            )
                        st = stage_p.tile([128, 512], bf16, name="stg",
                                          tag="stage")
                        nc.vector.tensor_copy(st[:], ps_o[:])
                        nc.sync.dma_start(
                            out_d.ap()[m * 128:(m + 1) * 128, qsl], st[:])
                    return emit

                for sc in range(12, NK):
                    weave_q.append(mk_v_unit(sc))
                # qkT[0] (mt0) completes in group 0, qkT[4] (mt2) in group
                # 1 -> rope [0,4] after g1, first used g4; qkT[1]/qkT[5]
                # by group 3 -> rope [1,5] after g3, first used g5.
                for mt in [0, 2, 1, 3]:
                    for n in range(NQ):
                        weave_q.append(mk_dec_unit(mt, n))

                def emit_group(qc, h, wcap):
                    # one (q-chunk, head) attention block; pops at most
                    # `wcap` weave units. scores run 2 kc ahead of av;
                    # weave slots precede the scores so the group never
                    # leads with a dependent matmul.
                    qsl = slice(qc * 512, (qc + 1) * 512)
                    ps_ctx = ps_c_p.tile([128, 512], f32, name="psc",
                                         tag="psc")
                    exps = []
                    dlvl1 = []
                    dlvl2 = []
                    budget = [wcap]

                    def weave_b(n=1):
                        for _ in range(n):
                            if weave_q and budget[0] > 0:
                                budget[0] -= 1
                                weave_q.pop(0)()

                    def emit_scores(kc):
                        ps_s = ps_s_p.tile([128, 512], f32, name="pss",
                                           tag="pss")
                        nc.tensor.matmul(
                            ps_s[:],
                            qkT[4 + h][:, kc * 128:(kc + 1) * 128],
                            qkT[h][:, qsl],
                            start=True, stop=True,
                        )
                        expT = exp_p.tile([128, 512], bf16, name="expT",
                                          tag="exp")
                        nc.scalar.activation(
                            expT[:], ps_s[:],
                            mybir.ActivationFunctionType.Exp, scale=SCALE)
                        exps.append(expT)
                        if kc % 2 == 1:
                            # lvl1 in bf16: 2x DVE rate; the rounding
                            # averages out over the 8 partial sums
                            d = den1_p.tile([128, 512], bf16, name="d1",
                                            tag="d1")
                            nc.vector.tensor_add(d[:], exps[kc - 1][:],
                                                 exps[kc][:])
                            dlvl1.append(d)
                            if kc % 4 == 3:
                                j2 = kc // 4
                                d2 = den2_p.tile([128, 512], f32,
                                                 name="d2", tag="d2")
                                nc.vector.tensor_add(
                                    d2[:], dlvl1[j2 * 2][:],
                                    dlvl1[j2 * 2 + 1][:])
                                dlvl2.append(d2)

                    def emit_av(kc):
                        nc.tensor.matmul(
                            ps_ctx[:],
                            v_nat[kc][:, h * 128:(h + 1) * 128],
                            exps[kc][:],
                            start=(kc == 0), stop=(kc == NK - 1),
                        )

                    weave_b()
                    emit_scores(0)
                    weave_b()
                    emit_scores(1)
                    weave_b()
                    for kc in range(2, NK):
                        emit_scores(kc)
                        emit_av(kc - 2)
                        weave_b()
                    emit_av(NK - 2)
                    weave_b()
                    emit_av(NK - 1)
                    # eager den tree: DVE starts it as soon as dlvl2[3]
                    # lands; two weave slots cover the remaining latency.
                    d3a = den3_p.tile([128, 512], f32, name="d3a", tag="d3")
                    nc.vector.tensor_add(d3a[:], dlvl2[0][:], dlvl2[1][:])
                    d3b = den3_p.tile([128, 512], f32, name="d3b", tag="d3")
                    nc.vector.tensor_add(d3b[:], dlvl2[2][:], dlvl2[3][:])
                    acc = acc_p.tile([128, 512], bf16, name="acc", tag="acc")
                    nc.vector.tensor_add(acc[:], d3a[:], d3b[:])
                    weave_b(2)
                    ps_den = ps_o_p.tile([128, 512], f32, name="psd",
                                         tag="pso")
                    nc.tensor.matmul(ps_den[:], ones_bf[:], acc[:],
                                     start=True, stop=True)
                    rden = rden_p.tile([128, 512], f32, name="rden",
                                       tag="rden")
                    nc.vector.reciprocal_approx_fast(rden[:], ps_den[:])
                    c_t = ctx_p.tile([128, 512], bf16, name="ctxt",
                                     tag="ctx")
                    nc.vector.tensor_mul(c_t[:], ps_ctx[:], rden[:])
                    if phases == 5:
                        r0 = (qc * 4 + h) * 128
                        nc.sync.dma_start(out_d.ap()[r0:r0 + 128, 0:512],
                                          c_t[:])
                    return c_t

                # x-projection heads first; dec-head groups after their
                # woven decompression + rope. Out-proj units at 5/group
                # keep groups 6+ PE-bound without bursts.
                order = [(0, 2), (0, 3), (1, 2), (1, 3), (0, 0), (0, 1),
                         (1, 0), (1, 1),
                         (2, 2), (2, 3), (2, 0), (2, 1),
                         (3, 2), (3, 3), (3, 0), (3, 1)]
                caps = [8, 4, 4, 4] + [5] * 12
                ctxs = {}
                for gi, (qc, h) in enumerate(order):
                    ctxs.setdefault(qc, {})[h] = emit_group(qc, h, caps[gi])
                    if gi == 1:
                        rope_pair(0, 4)
                    if gi == 3:
                        rope_pair(1, 5)
                    if len(ctxs[qc]) == 4:
                        dct = ctxs.pop(qc)
                        for m in range(16):
                            weave_q.append(mk_out_unit(qc, dct, m))
                while weave_q:
                    weave()
            lat_cm.__exit__(None, None, None)
            wdec_cm.__exit__(None, None, None)
            scr_cm.__exit__(None, None, None)
            swp_cm.__exit__(None, None, None)
            consts_cm.__exit__(None, None, None)

    nc.compile()
    return nc


def _get_program():
    if "nc" not in _prog_cache:
        _prog_cache["nc"] = _build_program()
    return _prog_cache["nc"]


def _host_shards(x, W_comp, W_q_dec, W_k_dec, W_v_dec, W_rope_q, W_rope_k,
                 W_out):
    import ml_dtypes
    bf16 = ml_dtypes.bfloat16

    inv = 1.0 / (10000.0 ** (np.arange(0, RD, 2, dtype=np.float32) / RD))
    ang = np.arange(S, dtype=np.float32)[:, None] * inv[None, :]     # [S, 32]
    cosT = np.cos(ang).T.astype(np.float32)                          # [32, S]
    sinT = np.sin(ang).T.astype(np.float32)
    cos4 = np.ascontiguousarray(np.tile(cosT, (4, 1))).astype(bf16)  # [128,S]
    sin4w = np.ascontiguousarray(np.tile(
        np.concatenate([-sinT, sinT], axis=0), (2, 1))).astype(bf16)  # [128,S]

    in_maps = []
    for c in range(NC):
        b, hg = divmod(c, 4)
        xTb = np.ascontiguousarray(x[b].T.astype(bf16))
        w_big = np.ascontiguousarray(np.concatenate(
            [W_comp,
             W_rope_q[:, hg * 256:(hg + 1) * 256],
             W_rope_k[:, hg * 256:(hg + 1) * 256]], axis=1).astype(bf16))
        w_qk = np.ascontiguousarray(np.concatenate(
            [W_q_dec[:, hg * 256:(hg + 1) * 256],
             W_k_dec[:, hg * 256:(hg + 1) * 256]], axis=1).astype(bf16))
        w_v = np.ascontiguousarray(np.concatenate(
            [W_v_dec[:, hg * 256:(hg + 1) * 256],
             W_v_dec[:, 1024 + hg * 256:1024 + (hg + 1) * 256]],
            axis=1).astype(bf16))
        w_o = np.ascontiguousarray(np.concatenate(
            [W_out[hg * 256:(hg + 1) * 256, :],
             W_out[1024 + hg * 256:1024 + (hg + 1) * 256, :]],
            axis=0).astype(bf16))
        in_maps.append({
            "xT": xTb, "w_big": w_big, "w_qk": w_qk, "w_v": w_v, "w_o": w_o,
            "cos4": cos4, "sin4w": sin4w,
        })
    return in_maps


def kernel(x, W_comp, W_q_dec, W_k_dec, W_v_dec, W_rope_q, W_rope_k, W_out,
           _trace=False):
    from concourse import bass_utils

    x = np.asarray(x, np.float32)
    args = [np.asarray(a, np.float32)
            for a in (W_comp, W_q_dec, W_k_dec, W_v_dec,
                      W_rope_q, W_rope_k, W_out)]
    in_maps = _host_shards(x, *args)
    nc = _get_program()
    res = bass_utils.run_bass_kernel_spmd(
        nc, in_maps, core_ids=list(range(NC)), trace=_trace)
    out = np.zeros((B, S, D), np.float32)
    for c in range(NC):
        b = c // 4
        out[b] += res.results[c]["out"].astype(np.float32).T
    if _trace:
        kernel.last_exec_ns = res.exec_time_ns
    return out
